# revision 3
# baseline (speedup 1.0000x reference)
"""Cross-attention (B=4, C=256, H=W=64) on 8 TRN2 NeuronCores.

Sharding: data-parallel over batch (4) x sequence-parallel over query dim
(2 halves of n=4096), one (batch, half) pair per core. Full n x n attention
stays on-core; softmax rows (over keys j) are complete locally.

Per-core math, all layouts chosen so no large transpose is ever needed:
  Q[c,i]  = WqT.T @ x1_half + bq          (f32r matmuls, ACT bias-add)
  K[c,j]  = WkT.T @ x2 + bk
  Vt[j,c] = x2_chunk.T @ WvT              (V transposed "for free")
  St[j,i] = K_chunk.T @ Q                 (K chunk stationary)
  Et      = exp(St - C0)                  (ACT, f32r out; C0 global shift,
                                           cancels exactly in normalization)
  O[c,i] += Vt_chunk.T @ Et               (accumulated over j in PSUM)
  sumE[i] = ones.T @ (sum_chunks Et)      (DVE accumulate + M=1 matmul)
  out     = O * (1/sumE) + bv             (K=1 broadcast matmul + DVE/ACT)

float32r (11-bit mantissa, full PE rate for N>=256) is used for all big
matmuls; fp32 for the tiny normalization matmuls.
"""
import numpy as np

import concourse.bacc as bacc
import concourse.mybir as mybir
import concourse.tile as tile
from concourse.bass_utils import run_bass_kernel_spmd

B, C, H, W = 4, 256, 64, 64
N = H * W                 # 4096 keys per sample
NQ = N // 2               # 2048 queries per core
CC = C // 128             # 2 channel chunks
NJ = N // 128             # 32 key chunks
IB = 2                    # i-blocks of 1024 queries
IBS = NQ // IB            # 1024
C0 = 72.0                 # global softmax shift (see module docstring)

F32 = mybir.dt.float32
F32R = mybir.dt.float32r
EXP = mybir.ActivationFunctionType.Exp
IDENT = mybir.ActivationFunctionType.Identity

_CACHED = {}


def _build():
    nc = bacc.Bacc()
    x1s = nc.dram_tensor("x1s", [C, NQ], F32R, kind="ExternalInput")
    x2 = nc.dram_tensor("x2", [C, N], F32R, kind="ExternalInput")
    wqT = nc.dram_tensor("wqT", [C, C], F32R, kind="ExternalInput")
    wkT = nc.dram_tensor("wkT", [C, C], F32R, kind="ExternalInput")
    wvT = nc.dram_tensor("wvT", [C, C], F32R, kind="ExternalInput")
    bq = nc.dram_tensor("bq", [C, 1], F32, kind="ExternalInput")
    bk = nc.dram_tensor("bk", [C, 1], F32, kind="ExternalInput")
    bv = nc.dram_tensor("bv", [C, 1], F32, kind="ExternalInput")
    out = nc.dram_tensor("o", [C, NQ], F32, kind="ExternalOutput")

    with tile.TileContext(nc) as tc:
        with (
            tc.tile_pool(name="singles", bufs=1) as singles,
            tc.tile_pool(name="epool", bufs=3) as epool,
            tc.tile_pool(name="accp", bufs=2) as accp,
            tc.tile_pool(name="ep1", bufs=2) as ep1,
            tc.tile_pool(name="ep2", bufs=2) as ep2,
            tc.tile_pool(name="ps_s", bufs=2, space="PSUM") as ps_s,  # S/proj/epilogue (4 banks)
            tc.tile_pool(name="ps_o", bufs=1, space="PSUM") as ps_o,  # O accum (4 banks)
        ):
            # ---------------- constant / input loads ----------------
            w_q = singles.tile([128, CC, CC, 128], F32R)
            w_k = singles.tile([128, CC, CC, 128], F32R)
            w_v = singles.tile([128, CC, CC, 128], F32R)
            for t, d in ((w_q, wqT), (w_k, wkT), (w_v, wvT)):
                nc.sync.dma_start(
                    out=t, in_=d.ap().rearrange("(ci k) (co m) -> k ci co m", k=128, m=128)
                )
            b_q = singles.tile([128, CC, 1], F32)
            b_k = singles.tile([128, CC, 1], F32)
            b_v = singles.tile([128, CC, 1], F32)
            for t, d in ((b_q, bq), (b_k, bk), (b_v, bv)):
                nc.sync.dma_start(out=t, in_=d.ap().rearrange("(cc c) x -> c cc x", c=128))

            x1_t = singles.tile([128, CC, NQ], F32R)
            nc.sync.dma_start(out=x1_t, in_=x1s.ap().rearrange("(cc c) n -> c cc n", c=128))
            x2_t = singles.tile([128, CC, N], F32R)
            nc.sync.dma_start(out=x2_t, in_=x2.ap().rearrange("(cc c) n -> c cc n", c=128))

            ones_j_f = singles.tile([128, 1], F32)
            nc.vector.memset(ones_j_f, 1.0)
            ones_m = singles.tile([1, 128], F32)
            nc.vector.memset(ones_m, 1.0)
            negc0 = singles.tile([128, 1], F32)
            nc.vector.memset(negc0, -C0)

            # ---------------- projections ----------------
            q_t = singles.tile([128, CC, NQ], F32R)
            k_t = singles.tile([128, CC, N], F32R)
            for dst, w, b, x, n in (
                (q_t, w_q, b_q, x1_t, NQ),
                (k_t, w_k, b_k, x2_t, N),
            ):
                for co in range(CC):
                    for nb in range(n // 512):
                        ps = ps_s.tile([128, 512], F32, tag="s", name="ps")
                        sl = slice(nb * 512, (nb + 1) * 512)
                        for ci in range(CC):
                            nc.tensor.matmul(
                                ps, lhsT=w[:, ci, co, :], rhs=x[:, ci, sl],
                                start=(ci == 0), stop=(ci == CC - 1),
                            )
                        nc.scalar.activation(dst[:, co, sl], ps, IDENT, bias=b[:, co, :])

            # V^T: [j, c] tiles, one 128-row chunk per jc
            v_t = singles.tile([128, NJ, C], F32R)
            for jc in range(NJ):
                ps = ps_s.tile([128, C], F32, tag="s", name="ps")
                jsl = slice(jc * 128, (jc + 1) * 128)
                for ci in range(CC):
                    nc.tensor.matmul(
                        ps, lhsT=x2_t[:, ci, jsl], rhs=w_v[:, ci, :, :],
                        start=(ci == 0), stop=(ci == CC - 1),
                    )
                nc.vector.tensor_copy(v_t[:, jc, :], ps)

            # ---------------- attention ----------------
            for ib in range(IB):
                isl_all = slice(ib * IBS, (ib + 1) * IBS)
                o_ps = [
                    [ps_o.tile([128, 512], F32, tag=f"o{cc}{h}", name=f"ops{cc}{h}") for h in range(2)]
                    for cc in range(CC)
                ]
                acc = accp.tile([128, IBS], F32, tag="acc")
                for jc in range(NJ):
                    jsl = slice(jc * 128, (jc + 1) * 128)
                    s_ps = ps_s.tile([128, IBS], F32, tag="s")
                    for h in range(2):
                        hsl = slice(h * 512, (h + 1) * 512)
                        qsl = slice(ib * IBS + h * 512, ib * IBS + (h + 1) * 512)
                        for ci in range(CC):
                            nc.tensor.matmul(
                                s_ps[:, hsl], lhsT=k_t[:, ci, jsl], rhs=q_t[:, ci, qsl],
                                start=(ci == 0), stop=(ci == CC - 1),
                            )
                    e_t = epool.tile([128, IBS], F32R, tag="e")
                    nc.scalar.activation(e_t, s_ps, EXP, bias=negc0, scale=1.0)
                    if jc == 0:
                        nc.vector.tensor_copy(acc, e_t)
                    else:
                        nc.vector.tensor_add(acc, acc, e_t)
                    for cc in range(CC):
                        csl = slice(cc * 128, (cc + 1) * 128)
                        for h in range(2):
                            hsl = slice(h * 512, (h + 1) * 512)
                            nc.tensor.matmul(
                                o_ps[cc][h], lhsT=v_t[:, jc, csl], rhs=e_t[:, hsl],
                                start=(jc == 0), stop=(jc == NJ - 1),
                            )

                # ---- epilogue: normalize + bias + store ----
                se_ps = ps_s.tile([1, IBS], F32, tag="s")
                for h in range(2):
                    hsl = slice(h * 512, (h + 1) * 512)
                    nc.tensor.matmul(
                        se_ps[:, hsl], lhsT=ones_j_f, rhs=acc[:, hsl],
                        start=True, stop=True,
                    )
                r_sb = ep1.tile([1, IBS], F32, tag="r")
                nc.vector.reciprocal(r_sb, se_ps)
                rb_ps = ps_s.tile([128, IBS], F32, tag="s")
                for h in range(2):
                    hsl = slice(h * 512, (h + 1) * 512)
                    nc.tensor.matmul(
                        rb_ps[:, hsl], lhsT=ones_m, rhs=r_sb[:, hsl],
                        start=True, stop=True,
                    )
                rb_sb = ep1.tile([128, IBS], F32, tag="rb")
                nc.vector.tensor_copy(rb_sb, rb_ps)
                for cc in range(CC):
                    for h in range(2):
                        hsl = slice(h * 512, (h + 1) * 512)
                        o1 = ep2.tile([128, 512], F32, tag="o1")
                        nc.vector.tensor_mul(o1, o_ps[cc][h], rb_sb[:, hsl])
                        o2 = ep2.tile([128, 512], F32, tag="o2")
                        nc.scalar.activation(o2, o1, IDENT, bias=b_v[:, cc, :])
                        nc.sync.dma_start(
                            out=out[cc * 128:(cc + 1) * 128,
                                    ib * IBS + h * 512: ib * IBS + (h + 1) * 512],
                            in_=o2,
                        )
    nc.compile()
    return nc


def kernel(x1, x2, Wq, bq, Wk, bk, Wv, bv):
    x1 = np.ascontiguousarray(np.asarray(x1, dtype=np.float32)).reshape(B, C, N)
    x2 = np.ascontiguousarray(np.asarray(x2, dtype=np.float32)).reshape(B, C, N)
    wqT = np.ascontiguousarray(np.asarray(Wq, dtype=np.float32).T)
    wkT = np.ascontiguousarray(np.asarray(Wk, dtype=np.float32).T)
    wvT = np.ascontiguousarray(np.asarray(Wv, dtype=np.float32).T)
    bq = np.asarray(bq, dtype=np.float32).reshape(C, 1)
    bk = np.asarray(bk, dtype=np.float32).reshape(C, 1)
    bv = np.asarray(bv, dtype=np.float32).reshape(C, 1)

    if "nc" not in _CACHED:
        _CACHED["nc"] = _build()
    nc = _CACHED["nc"]

    in_maps = []
    for core in range(8):
        b, half = divmod(core, 2)
        in_maps.append({
            "x1s": np.ascontiguousarray(x1[b][:, half * NQ:(half + 1) * NQ]),
            "x2": x2[b],
            "wqT": wqT, "wkT": wkT, "wvT": wvT,
            "bq": bq, "bk": bk, "bv": bv,
        })
    res = run_bass_kernel_spmd(nc, in_maps, core_ids=list(range(8)))
    out = np.empty((B, C, N), dtype=np.float32)
    for core in range(8):
        b, half = divmod(core, 2)
        out[b][:, half * NQ:(half + 1) * NQ] = res.results[core]["o"]
    return out.reshape(B, C, H, W)
